# revision 19
# baseline (speedup 1.0000x reference)
"""Bidirectional LSTM (B=64, L=512, I=512, H=768) on 8 Trainium2 NeuronCores.

Sharding: communication-free. 8 cores = 2 directions x 4 batch-quarters.
Each core runs the full recurrence of one direction on 16 sequences.

Per-core design:
  - Gate columns permuted (host-side) into 4 strips of 768 = [i|f|o|g] x 192.
    Output strip g lives on PSUM/SBUF partitions [32g, 32g+32) (16 real batch
    rows + 16 defined dummies), so elementwise ops use all 128 lanes.
  - Recurrence matmul: out strips via col-group tile_position (0, 32g), 4
    concurrent streams; moving = W_hh^T slices (bf16; 4-byte dtypes are
    rejected for tile_position matmuls), stationary = h^T chunks.
  - h -> h^T via ONE DVE 32x32 block transpose (PE transpose from partition
    base 32/64 crashes the device). The block-scattered h^T layout is
    matched by a host-side row scattering of W_hh^T so contraction chunks
    stay K=128: chunk c rows = dims {192 s + 32 c + j : s, j}.
  - xg_t + bias injected into PSUM via a K=17 identity matmul
    (lhsT = [I_16 pad; ones], rhs = [xg_t; bias]).
  - Input projection xg = x @ W_ih^T runs as an fp32r M=128 GEMM (full FP22
    precision), result stored bf16 in HBM, interleaved into recurrence
    bubbles.
"""

import numpy as np
import ml_dtypes
from contextlib import ExitStack

import concourse.bass as bass
import concourse.bacc as bacc
import concourse.tile as tile
import concourse.mybir as mybir
from concourse.bass_utils import run_bass_kernel_spmd

AF = mybir.ActivationFunctionType
F32 = mybir.dt.float32
F32R = mybir.dt.float32r
BF16 = mybir.dt.bfloat16

B, L, I, H = 64, 512, 512, 768
G4 = 4 * H
NSTRIP = 4
WS = G4 // NSTRIP            # 768 gate cols per strip
HS = H // NSTRIP             # 192 h dims per strip
NC6 = HS // 32               # 6 32-blocks per strip = contraction chunks
BC = B // 4                  # 16 sequences per core
KI = I // 128                # 4 contraction chunks for the input GEMM
SIG = 3 * HS                 # 576: sigmoid region width per strip
GEMM_AHEAD_TILES = 8         # GEMM M-tiles (8 timesteps each) emitted up front

# column permutation: ours -> original gate index
_PERM = np.zeros(G4, dtype=np.int64)
_BASE = {0: 0, 1: H, 2: 3 * H, 3: 2 * H}    # i, f, o, g
for _s in range(NSTRIP):
    for _blk in range(4):
        for _j in range(HS):
            _PERM[_s * WS + _blk * HS + _j] = _BASE[_blk] + HS * _s + _j


def build_nc(L_steps=L, interleave=True, stage=5):
    assert L_steps % 8 == 0
    nc = bacc.Bacc("TRN2", target_bir_lowering=False, debug=False)

    xT = nc.dram_tensor("xT", [I, L_steps, BC], F32R, kind="ExternalInput").ap()
    wihT = nc.dram_tensor("wihT", [I, G4], F32R, kind="ExternalInput").ap()
    whhS = nc.dram_tensor("whhS", [128, NC6, G4], BF16, kind="ExternalInput").ap()
    biasd = nc.dram_tensor("biasd", [1, G4], BF16, kind="ExternalInput").ap()
    h0S = nc.dram_tensor("h0S", [128, HS], BF16, kind="ExternalInput").ap()
    c0s = nc.dram_tensor("c0s", [NSTRIP, BC, HS], F32, kind="ExternalInput").ap()
    augd = nc.dram_tensor("augd", [17, 32], BF16, kind="ExternalInput").ap()
    hout = nc.dram_tensor("hout", [L_steps, BC, H], F32, kind="ExternalOutput").ap()
    cnout = nc.dram_tensor("cnout", [NSTRIP, BC, HS], F32, kind="ExternalOutput").ap()

    MT = L_steps // 8            # GEMM M-tiles (128 rows = 8 t x 16 b)

    with ExitStack() as ctx:
        tc = ctx.enter_context(tile.TileContext(nc))
        wts = ctx.enter_context(tc.tile_pool(name="wts", bufs=1))
        state = ctx.enter_context(tc.tile_pool(name="state", bufs=1))
        xgp = ctx.enter_context(tc.tile_pool(name="xgp", bufs=3))
        work = ctx.enter_context(tc.tile_pool(name="work", bufs=2))
        hsp = ctx.enter_context(tc.tile_pool(name="hsp", bufs=2))
        gemmx = ctx.enter_context(tc.tile_pool(name="gemmx", bufs=3))
        gpsum = ctx.enter_context(tc.tile_pool(name="gpsum", bufs=2, space="PSUM"))
        gemmp = ctx.enter_context(tc.tile_pool(name="gemmp", bufs=2, space="PSUM"))
        dram = ctx.enter_context(tc.tile_pool(name="dram", bufs=1, space="DRAM"))

        # resident tensors
        wih_sb = wts.tile([128, KI, G4], F32R)
        whh_sb = wts.tile([128, NC6, G4], BF16)
        aug = wts.tile([17, 32], BF16)       # [I_16 pad to 32 ; ones]
        c = state.tile([128, HS], F32)       # cell state, strip s at rows 32s..

        nc.sync.dma_start(wih_sb[:], wihT.rearrange("(k p) g -> p k g", p=128))
        nc.sync.dma_start(whh_sb[:], whhS)
        nc.sync.dma_start(aug[:], augd)
        nc.vector.memset(c[:], 0.0)
        for s in range(NSTRIP):
            nc.sync.dma_start(c[32 * s:32 * s + BC, :], c0s[s])

        hTs = hsp.tile([128, HS], BF16, tag="hTs")   # block-scattered h^T
        nc.sync.dma_start(hTs[:], h0S)

        xg_dram = dram.tile([L_steps, BC, G4], BF16)

        # ---------------- phase A: input-projection GEMM ----------------
        gemm_jobs = []
        xt_tiles = {}

        def emit_gemm_block(m, n):
            if n == 0:
                t0 = 8 * m
                xt = gemmx.tile([128, KI, 128], F32R)
                nc.sync.dma_start(
                    xt[:], xT[:, t0:t0 + 8, :].rearrange("(k p) t b -> p k (t b)", p=128))
                xt_tiles[m] = xt
            xt = xt_tiles[m]
            ps = gemmp.tile([128, 512], F32)
            for k in range(KI):
                nc.tensor.matmul(ps[:], xt[:, k, :],
                                 wih_sb[:, k, 512 * n:512 * (n + 1)],
                                 start=(k == 0), stop=(k == KI - 1))
            xgs = gemmx.tile([128, 512], BF16, tag="xgs")
            nc.any.tensor_copy(xgs[:], ps[:])
            nc.sync.dma_start(
                xg_dram[8 * m:8 * (m + 1), :, 512 * n:512 * (n + 1)], xgs[:])
            if n == G4 // 512 - 1:
                del xt_tiles[m]

        for m in range(MT):
            for n in range(G4 // 512):
                gemm_jobs.append((m, n))
        gemm_pos = 0

        def pump_gemm(target):
            nonlocal gemm_pos
            while gemm_pos < min(target, len(gemm_jobs)):
                emit_gemm_block(*gemm_jobs[gemm_pos])
                gemm_pos += 1

        nblk = G4 // 512
        if interleave:
            pump_gemm(GEMM_AHEAD_TILES * nblk)
        else:
            pump_gemm(len(gemm_jobs))

        # ---------------- phase B: recurrence ----------------
        for t in range(L_steps if stage >= 1 else 0):
            xg = xgp.tile([17, G4], BF16)
            nc.sync.dma_start(xg[0:BC, :], xg_dram[t])
            nc.sync.dma_start(xg[BC:BC + 1, :], biasd)

            g_ps = gpsum.tile([128, WS], F32)
            # xg+bias injection; each (strip g, nb-bank) accumulation group
            # opens with start=True here.  nb0: cols 0:512, nb1: 512:768.
            for (o0, nn) in [(0, 512), (512, 256)]:
                for g in range(NSTRIP):
                    nc.tensor.matmul(
                        g_ps[32 * g:32 * g + 32, o0:o0 + nn],
                        aug[:], xg[:, WS * g + o0:WS * g + o0 + nn],
                        start=True, stop=False,
                        tile_position=(0, 32 * g), skip_group_check=True)
            for cc in range(NC6 if stage >= 2 else 0):
                for (o0, nn) in [(0, 512), (512, 256)]:
                    for g in range(NSTRIP):
                        nc.tensor.matmul(
                            g_ps[32 * g:32 * g + 32, o0:o0 + nn],
                            hTs[:, 32 * cc:32 * cc + 32],
                            whh_sb[:, cc, WS * g + o0:WS * g + o0 + nn],
                            start=False, stop=(cc == NC6 - 1),
                            tile_position=(0, 32 * g), skip_group_check=True)

            if stage < 3:
                dg = work.tile([128, WS], F32, tag="dg")
                nc.vector.tensor_copy(dg[:, :], g_ps[:, :])
                for s in range(NSTRIP):
                    nc.sync.dma_start(hout[t, :, HS * s:HS * (s + 1)],
                                      dg[32 * s:32 * s + BC, 0:HS])
                continue

            sg = work.tile([128, SIG], F32)
            tg = work.tile([128, HS], F32)
            tc_ = work.tile([128, HS], F32)
            t1 = work.tile([128, HS], F32)
            h = work.tile([128, HS], F32)
            nc.scalar.activation(sg[:, :], g_ps[:, 0:SIG], AF.Sigmoid)
            nc.scalar.activation(tg[:, :], g_ps[:, SIG:WS], AF.Tanh)
            nc.vector.tensor_mul(c[:, :], sg[:, HS:2 * HS], c[:, :])
            nc.vector.tensor_mul(t1[:, :], sg[:, 0:HS], tg[:, :])
            nc.vector.tensor_add(c[:, :], c[:, :], t1[:, :])
            nc.scalar.activation(tc_[:, :], c[:, :], AF.Tanh)
            nc.vector.tensor_mul(h[:, :], sg[:, 2 * HS:SIG], tc_[:, :])

            if stage >= 4:
                hb = work.tile([128, HS], BF16)
                nc.vector.tensor_copy(hb[:, :], h[:, :])
                hTs = hsp.tile([128, HS], BF16, tag="hTs")
                nc.vector.transpose(hTs[:, :], hb[:, :])

            for s in range(NSTRIP):
                nc.sync.dma_start(hout[t, :, HS * s:HS * (s + 1)],
                                  h[32 * s:32 * s + BC, :])
            if t == L_steps - 1:
                for s in range(NSTRIP):
                    nc.sync.dma_start(cnout[s], c[32 * s:32 * s + BC, :])

            if interleave:
                pump_gemm(GEMM_AHEAD_TILES * nblk + ((t + 1) * nblk * MT) // L_steps)
        pump_gemm(len(gemm_jobs))

    nc.compile()
    return nc


def make_in_maps(x, weight_ih, weight_hh, bias, weight_ih_reverse,
                 weight_hh_reverse, bias_reverse, h0, c0, L_steps=L):
    """Per-core input dicts. Core c: direction c//4, batch quarter c%4."""
    in_maps = []
    for core in range(8):
        d, q = core // 4, core % 4
        bsl = slice(BC * q, BC * (q + 1))
        xq = np.asarray(x[bsl, :L_steps], dtype=np.float32)
        if d == 1:
            xq = xq[:, ::-1, :]
        Wih = weight_ih if d == 0 else weight_ih_reverse
        Whh = weight_hh if d == 0 else weight_hh_reverse
        bb = bias if d == 0 else bias_reverse
        wihT_p = np.ascontiguousarray(np.asarray(Wih, np.float32).T[:, _PERM])
        whhT_p = np.asarray(Whh, np.float32).T[:, _PERM]     # (H, G4), rows = dims
        # scatter rows: whhS[32 s + j, c, :] = whhT_p[192 s + 32 c + j, :]
        whhS = np.ascontiguousarray(
            whhT_p.reshape(NSTRIP, NC6, 32, G4).transpose(0, 2, 1, 3)
            .reshape(128, NC6, G4)).astype(ml_dtypes.bfloat16)
        bias_p = np.ascontiguousarray(
            np.asarray(bb, np.float32)[_PERM])[None, :].astype(ml_dtypes.bfloat16)
        h0q = np.asarray(h0[d, bsl], np.float32)             # (BC, H)
        c0q = np.asarray(c0[d, bsl], np.float32)
        # initial block-scattered h^T: h0S[32 s + j, 32 c + b] = h0q[b, 192 s + 32 c + j]
        h0S = np.zeros((NSTRIP, 32, NC6, 32), np.float32)
        h0r = h0q.reshape(BC, NSTRIP, NC6, 32)               # [b, s, c, j]
        h0S[:, :, :, 0:BC] = h0r.transpose(1, 3, 2, 0)       # [s, j, c, b]
        h0S = h0S.reshape(128, HS).astype(ml_dtypes.bfloat16)
        c0_strips = np.ascontiguousarray(
            c0q.reshape(BC, NSTRIP, HS).transpose(1, 0, 2))  # (NSTRIP, BC, HS)
        aug_np = np.zeros((17, 32), ml_dtypes.bfloat16)
        aug_np[0:16, 0:BC] = np.eye(BC, dtype=np.float32)
        aug_np[16, :] = 1.0
        in_maps.append({
            "xT": np.ascontiguousarray(xq.transpose(2, 1, 0)),  # (I, L, BC)
            "wihT": wihT_p,
            "whhS": whhS,
            "biasd": bias_p,
            "h0S": h0S,
            "c0s": c0_strips,
            "augd": aug_np,
        })
    return in_maps


def assemble(results, L_steps=L):
    out_f = np.zeros((B, L_steps, H), np.float32)
    out_r = np.zeros((B, L_steps, H), np.float32)
    hn = np.zeros((2, B, H), np.float32)
    cn = np.zeros((2, B, H), np.float32)
    for core in range(8):
        d, q = core // 4, core % 4
        bsl = slice(BC * q, BC * (q + 1))
        hseq = results[core]["hout"].transpose(1, 0, 2)      # (BC, L, H)
        cfin = results[core]["cnout"].transpose(1, 0, 2).reshape(BC, H)
        if d == 0:
            out_f[bsl] = hseq
            hn[0, bsl] = hseq[:, -1, :]
        else:
            out_r[bsl] = hseq[:, ::-1, :]
            hn[1, bsl] = hseq[:, -1, :]
        cn[d, bsl] = cfin
    h_all = np.concatenate([out_f, out_r], axis=-1)
    return h_all, hn, cn


_NC_CACHE = {}


def _get_nc(L_steps=L, interleave=True):
    key = (L_steps, interleave)
    if key not in _NC_CACHE:
        _NC_CACHE[key] = build_nc(L_steps, interleave)
    return _NC_CACHE[key]


def kernel(x, weight_ih, weight_hh, bias, weight_ih_reverse,
           weight_hh_reverse, bias_reverse, h0, c0, _trace=False, _tmpdir=None):
    nc = _get_nc()
    in_maps = make_in_maps(x, weight_ih, weight_hh, bias, weight_ih_reverse,
                           weight_hh_reverse, bias_reverse, h0, c0)
    res = run_bass_kernel_spmd(nc, in_maps, core_ids=list(range(8)),
                               trace=_trace, tmpdir=_tmpdir)
    out = assemble(res.results)
    kernel.last_results = res
    return out


# revision 21
# speedup vs baseline: 1.0035x; 1.0035x over previous
"""Bidirectional LSTM (B=64, L=512, I=512, H=768) on 8 Trainium2 NeuronCores.

Sharding: communication-free. 8 cores = 2 directions x 4 batch-quarters.
Each core runs the full recurrence of one direction on 16 sequences.

Per-core design:
  - Gate columns permuted (host-side) into 4 strips of 768 = [i|f|o|g] x 192.
    Output strip g lives on PSUM/SBUF partitions [32g, 32g+32) (16 real batch
    rows + 16 defined dummies), so elementwise ops use all 128 lanes.
  - Recurrence matmul: out strips via col-group tile_position (0, 32g), 4
    concurrent streams; moving = W_hh^T slices (bf16; 4-byte dtypes are
    rejected for tile_position matmuls), stationary = h^T chunks.
  - h -> h^T via ONE DVE 32x32 block transpose (PE transpose from partition
    base 32/64 crashes the device). The block-scattered h^T layout is
    matched by a host-side row scattering of W_hh^T so contraction chunks
    stay K=128: chunk c rows = dims {192 s + 32 c + j : s, j}.
  - xg_t + bias injected into PSUM via a K=17 identity matmul
    (lhsT = [I_16 pad; ones], rhs = [xg_t; bias]).
  - Input projection xg = x @ W_ih^T runs as an fp32r M=128 GEMM (full FP22
    precision), result stored bf16 in HBM, interleaved into recurrence
    bubbles.
"""

import numpy as np
import ml_dtypes
from contextlib import ExitStack

import concourse.bass as bass
import concourse.bacc as bacc
import concourse.tile as tile
import concourse.mybir as mybir
from concourse.bass_utils import run_bass_kernel_spmd

AF = mybir.ActivationFunctionType
F32 = mybir.dt.float32
F32R = mybir.dt.float32r
BF16 = mybir.dt.bfloat16

B, L, I, H = 64, 512, 512, 768
G4 = 4 * H
NSTRIP = 4
WS = G4 // NSTRIP            # 768 gate cols per strip
HS = H // NSTRIP             # 192 h dims per strip
NC6 = HS // 32               # 6 32-blocks per strip = contraction chunks
BC = B // 4                  # 16 sequences per core
KI = I // 128                # 4 contraction chunks for the input GEMM
SIG = 3 * HS                 # 576: sigmoid region width per strip
GEMM_AHEAD_TILES = 8         # GEMM M-tiles (8 timesteps each) emitted up front

# column permutation: ours -> original gate index
_PERM = np.zeros(G4, dtype=np.int64)
_BASE = {0: 0, 1: H, 2: 3 * H, 3: 2 * H}    # i, f, o, g
for _s in range(NSTRIP):
    for _blk in range(4):
        for _j in range(HS):
            _PERM[_s * WS + _blk * HS + _j] = _BASE[_blk] + HS * _s + _j


def build_nc(L_steps=L, interleave=True, stage=5):
    assert L_steps % 8 == 0
    nc = bacc.Bacc("TRN2", target_bir_lowering=False, debug=False)

    xT = nc.dram_tensor("xT", [I, L_steps, BC], F32R, kind="ExternalInput").ap()
    wihT = nc.dram_tensor("wihT", [I, G4], F32R, kind="ExternalInput").ap()
    whhS = nc.dram_tensor("whhS", [128, NC6, G4], BF16, kind="ExternalInput").ap()
    biasd = nc.dram_tensor("biasd", [1, G4], BF16, kind="ExternalInput").ap()
    h0S = nc.dram_tensor("h0S", [128, HS], BF16, kind="ExternalInput").ap()
    c0s = nc.dram_tensor("c0s", [NSTRIP, BC, HS], F32, kind="ExternalInput").ap()
    augd = nc.dram_tensor("augd", [17, 32], BF16, kind="ExternalInput").ap()
    hout = nc.dram_tensor("hout", [L_steps, BC, H], BF16, kind="ExternalOutput").ap()
    cnout = nc.dram_tensor("cnout", [NSTRIP, BC, HS], F32, kind="ExternalOutput").ap()

    MT = L_steps // 8            # GEMM M-tiles (128 rows = 8 t x 16 b)

    with ExitStack() as ctx:
        tc = ctx.enter_context(tile.TileContext(nc))
        wts = ctx.enter_context(tc.tile_pool(name="wts", bufs=1))
        state = ctx.enter_context(tc.tile_pool(name="state", bufs=1))
        xgp = ctx.enter_context(tc.tile_pool(name="xgp", bufs=3))
        work = ctx.enter_context(tc.tile_pool(name="work", bufs=2))
        hsp = ctx.enter_context(tc.tile_pool(name="hsp", bufs=2))
        gemmx = ctx.enter_context(tc.tile_pool(name="gemmx", bufs=3))
        gpsum = ctx.enter_context(tc.tile_pool(name="gpsum", bufs=2, space="PSUM"))
        gemmp = ctx.enter_context(tc.tile_pool(name="gemmp", bufs=2, space="PSUM"))
        dram = ctx.enter_context(tc.tile_pool(name="dram", bufs=1, space="DRAM"))

        # resident tensors
        wih_sb = wts.tile([128, KI, G4], F32R)
        whh_sb = wts.tile([128, NC6, G4], BF16)
        aug = wts.tile([17, 32], BF16)       # [I_16 pad to 32 ; ones]
        c = state.tile([128, HS], F32)       # cell state, strip s at rows 32s..

        nc.sync.dma_start(wih_sb[:], wihT.rearrange("(k p) g -> p k g", p=128))
        nc.sync.dma_start(whh_sb[:], whhS)
        nc.sync.dma_start(aug[:], augd)
        nc.vector.memset(c[:], 0.0)
        for s in range(NSTRIP):
            nc.sync.dma_start(c[32 * s:32 * s + BC, :], c0s[s])

        hTs = hsp.tile([128, HS], BF16, tag="hTs")   # block-scattered h^T
        nc.sync.dma_start(hTs[:], h0S)

        xg_dram = dram.tile([L_steps, BC, G4], BF16)

        # ---------------- phase A: input-projection GEMM ----------------
        gemm_jobs = []
        xt_tiles = {}

        def emit_gemm_block(m, n):
            if n == 0:
                t0 = 8 * m
                xt = gemmx.tile([128, KI, 128], F32R)
                nc.sync.dma_start(
                    xt[:], xT[:, t0:t0 + 8, :].rearrange("(k p) t b -> p k (t b)", p=128))
                xt_tiles[m] = xt
            xt = xt_tiles[m]
            ps = gemmp.tile([128, 512], F32)
            for k in range(KI):
                nc.tensor.matmul(ps[:], xt[:, k, :],
                                 wih_sb[:, k, 512 * n:512 * (n + 1)],
                                 start=(k == 0), stop=(k == KI - 1))
            xgs = gemmx.tile([128, 512], BF16, tag="xgs")
            nc.any.tensor_copy(xgs[:], ps[:])
            nc.sync.dma_start(
                xg_dram[8 * m:8 * (m + 1), :, 512 * n:512 * (n + 1)], xgs[:])
            if n == G4 // 512 - 1:
                del xt_tiles[m]

        for m in range(MT):
            for n in range(G4 // 512):
                gemm_jobs.append((m, n))
        gemm_pos = 0

        def pump_gemm(target):
            nonlocal gemm_pos
            while gemm_pos < min(target, len(gemm_jobs)):
                emit_gemm_block(*gemm_jobs[gemm_pos])
                gemm_pos += 1

        nblk = G4 // 512
        if interleave:
            pump_gemm(GEMM_AHEAD_TILES * nblk)
        else:
            pump_gemm(len(gemm_jobs))

        # ---------------- phase B: recurrence ----------------
        for t in range(L_steps if stage >= 1 else 0):
            xg = xgp.tile([17, G4], BF16)
            nc.sync.dma_start(xg[0:BC, :], xg_dram[t])
            nc.sync.dma_start(xg[BC:BC + 1, :], biasd)

            g_ps = gpsum.tile([128, 1024], F32)
            # PSUM banks: nb0 -> psum cols [0:384] = gate cols [0:384] (i, f);
            # nb1 -> psum cols [512:896] = gate cols [384:768] (o, g).
            # Each (strip g, bank) accumulation group opens with start=True.
            for (po, go, nn) in [(0, 0, 384), (512, 384, 384)]:
                for g in range(NSTRIP):
                    nc.tensor.matmul(
                        g_ps[32 * g:32 * g + 32, po:po + nn],
                        aug[:], xg[:, WS * g + go:WS * g + go + nn],
                        start=True, stop=False,
                        tile_position=(0, 32 * g), skip_group_check=True)
            for (po, go, nn) in [(0, 0, 384), (512, 384, 384)]:
                for cc in range(NC6 if stage >= 2 else 0):
                    for g in range(NSTRIP):
                        nc.tensor.matmul(
                            g_ps[32 * g:32 * g + 32, po:po + nn],
                            hTs[:, 32 * cc:32 * cc + 32],
                            whh_sb[:, cc, WS * g + go:WS * g + go + nn],
                            start=False, stop=(cc == NC6 - 1),
                            tile_position=(0, 32 * g), skip_group_check=True)
                if po == 0 and stage >= 3:
                    # sigmoid(i, f) while the o,g bank is still streaming
                    sg = work.tile([128, SIG], F32)
                    nc.scalar.activation(sg[:, 0:384], g_ps[:, 0:384], AF.Sigmoid)

            if stage < 3:
                dg = work.tile([128, WS], BF16, tag="dg")
                nc.vector.tensor_copy(dg[:, :], g_ps[:, 0:WS])
                for s in range(NSTRIP):
                    nc.sync.dma_start(hout[t, :, HS * s:HS * (s + 1)],
                                      dg[32 * s:32 * s + BC, 0:HS])
                continue

            # psum layout: i [0:192], f [192:384], o [512:704], g [704:896]
            tg = work.tile([128, HS], F32)
            tc_ = work.tile([128, HS], F32)
            t1 = work.tile([128, HS], F32)
            h = work.tile([128, HS], BF16)
            nc.scalar.activation(tg[:, :], g_ps[:, 704:896], AF.Tanh)
            nc.vector.tensor_mul(c[:, :], sg[:, 192:384], c[:, :])
            nc.vector.tensor_mul(t1[:, :], sg[:, 0:192], tg[:, :])
            nc.scalar.activation(sg[:, 384:SIG], g_ps[:, 512:704], AF.Sigmoid)
            nc.vector.tensor_add(c[:, :], c[:, :], t1[:, :])
            nc.scalar.activation(tc_[:, :], c[:, :], AF.Tanh)
            nc.vector.tensor_mul(h[:, :], sg[:, 384:SIG], tc_[:, :])

            if stage >= 4:
                hTs = hsp.tile([128, HS], BF16, tag="hTs")
                nc.vector.transpose(hTs[:, :], h[:, :])

            for s in range(NSTRIP):
                nc.sync.dma_start(hout[t, :, HS * s:HS * (s + 1)],
                                  h[32 * s:32 * s + BC, :])
            if t == L_steps - 1:
                for s in range(NSTRIP):
                    nc.sync.dma_start(cnout[s], c[32 * s:32 * s + BC, :])

            if interleave:
                pump_gemm(GEMM_AHEAD_TILES * nblk + ((t + 1) * nblk * MT) // L_steps)
        pump_gemm(len(gemm_jobs))

    nc.compile()
    return nc


def make_in_maps(x, weight_ih, weight_hh, bias, weight_ih_reverse,
                 weight_hh_reverse, bias_reverse, h0, c0, L_steps=L):
    """Per-core input dicts. Core c: direction c//4, batch quarter c%4."""
    in_maps = []
    for core in range(8):
        d, q = core // 4, core % 4
        bsl = slice(BC * q, BC * (q + 1))
        xq = np.asarray(x[bsl, :L_steps], dtype=np.float32)
        if d == 1:
            xq = xq[:, ::-1, :]
        Wih = weight_ih if d == 0 else weight_ih_reverse
        Whh = weight_hh if d == 0 else weight_hh_reverse
        bb = bias if d == 0 else bias_reverse
        wihT_p = np.ascontiguousarray(np.asarray(Wih, np.float32).T[:, _PERM])
        whhT_p = np.asarray(Whh, np.float32).T[:, _PERM]     # (H, G4), rows = dims
        # scatter rows: whhS[32 s + j, c, :] = whhT_p[192 s + 32 c + j, :]
        whhS = np.ascontiguousarray(
            whhT_p.reshape(NSTRIP, NC6, 32, G4).transpose(0, 2, 1, 3)
            .reshape(128, NC6, G4)).astype(ml_dtypes.bfloat16)
        bias_p = np.ascontiguousarray(
            np.asarray(bb, np.float32)[_PERM])[None, :].astype(ml_dtypes.bfloat16)
        h0q = np.asarray(h0[d, bsl], np.float32)             # (BC, H)
        c0q = np.asarray(c0[d, bsl], np.float32)
        # initial block-scattered h^T: h0S[32 s + j, 32 c + b] = h0q[b, 192 s + 32 c + j]
        h0S = np.zeros((NSTRIP, 32, NC6, 32), np.float32)
        h0r = h0q.reshape(BC, NSTRIP, NC6, 32)               # [b, s, c, j]
        h0S[:, :, :, 0:BC] = h0r.transpose(1, 3, 2, 0)       # [s, j, c, b]
        h0S = h0S.reshape(128, HS).astype(ml_dtypes.bfloat16)
        c0_strips = np.ascontiguousarray(
            c0q.reshape(BC, NSTRIP, HS).transpose(1, 0, 2))  # (NSTRIP, BC, HS)
        aug_np = np.zeros((17, 32), ml_dtypes.bfloat16)
        aug_np[0:16, 0:BC] = np.eye(BC, dtype=np.float32)
        aug_np[16, :] = 1.0
        in_maps.append({
            "xT": np.ascontiguousarray(xq.transpose(2, 1, 0)),  # (I, L, BC)
            "wihT": wihT_p,
            "whhS": whhS,
            "biasd": bias_p,
            "h0S": h0S,
            "c0s": c0_strips,
            "augd": aug_np,
        })
    return in_maps


def assemble(results, L_steps=L):
    out_f = np.zeros((B, L_steps, H), np.float32)
    out_r = np.zeros((B, L_steps, H), np.float32)
    hn = np.zeros((2, B, H), np.float32)
    cn = np.zeros((2, B, H), np.float32)
    for core in range(8):
        d, q = core // 4, core % 4
        bsl = slice(BC * q, BC * (q + 1))
        hseq = np.asarray(results[core]["hout"],
                          dtype=np.float32).transpose(1, 0, 2)  # (BC, L, H)
        cfin = results[core]["cnout"].transpose(1, 0, 2).reshape(BC, H)
        if d == 0:
            out_f[bsl] = hseq
            hn[0, bsl] = hseq[:, -1, :]
        else:
            out_r[bsl] = hseq[:, ::-1, :]
            hn[1, bsl] = hseq[:, -1, :]
        cn[d, bsl] = cfin
    h_all = np.concatenate([out_f, out_r], axis=-1)
    return h_all, hn, cn


_NC_CACHE = {}


def _get_nc(L_steps=L, interleave=True):
    key = (L_steps, interleave)
    if key not in _NC_CACHE:
        _NC_CACHE[key] = build_nc(L_steps, interleave)
    return _NC_CACHE[key]


def kernel(x, weight_ih, weight_hh, bias, weight_ih_reverse,
           weight_hh_reverse, bias_reverse, h0, c0, _trace=False, _tmpdir=None):
    nc = _get_nc()
    in_maps = make_in_maps(x, weight_ih, weight_hh, bias, weight_ih_reverse,
                           weight_hh_reverse, bias_reverse, h0, c0)
    res = run_bass_kernel_spmd(nc, in_maps, core_ids=list(range(8)),
                               trace=_trace, tmpdir=_tmpdir)
    out = assemble(res.results)
    kernel.last_results = res
    return out


# revision 22
# speedup vs baseline: 1.0347x; 1.0311x over previous
"""Bidirectional LSTM (B=64, L=512, I=512, H=768) on 8 Trainium2 NeuronCores.

Sharding: communication-free. 8 cores = 2 directions x 4 batch-quarters.
Each core runs the full recurrence of one direction on 16 sequences.

Per-core design:
  - Gate columns permuted (host-side) into 4 strips of 768 = [i|f|o|g] x 192.
    Output strip g lives on PSUM/SBUF partitions [32g, 32g+32) (16 real batch
    rows + 16 defined dummies), so elementwise ops use all 128 lanes.
  - Recurrence matmul: out strips via col-group tile_position (0, 32g), 4
    concurrent streams; moving = W_hh^T slices (bf16; 4-byte dtypes are
    rejected for tile_position matmuls), stationary = h^T chunks.
  - h -> h^T via ONE DVE 32x32 block transpose (PE transpose from partition
    base 32/64 crashes the device). The block-scattered h^T layout is
    matched by a host-side row scattering of W_hh^T so contraction chunks
    stay K=128: chunk c rows = dims {192 s + 32 c + j : s, j}.
  - xg_t + bias injected into PSUM via a K=17 identity matmul
    (lhsT = [I_16 pad; ones], rhs = [xg_t; bias]).
  - Input projection xg = x @ W_ih^T runs as an fp32r M=128 GEMM (full FP22
    precision), result stored bf16 in HBM, interleaved into recurrence
    bubbles.
"""

import numpy as np
import ml_dtypes
from contextlib import ExitStack

import concourse.bass as bass
import concourse.bacc as bacc
import concourse.tile as tile
import concourse.mybir as mybir
from concourse.bass_utils import run_bass_kernel_spmd

AF = mybir.ActivationFunctionType
F32 = mybir.dt.float32
F32R = mybir.dt.float32r
BF16 = mybir.dt.bfloat16

B, L, I, H = 64, 512, 512, 768
G4 = 4 * H
NSTRIP = 4
WS = G4 // NSTRIP            # 768 gate cols per strip
HS = H // NSTRIP             # 192 h dims per strip
NC6 = HS // 32               # 6 32-blocks per strip = contraction chunks
BC = B // 4                  # 16 sequences per core
KI = I // 128                # 4 contraction chunks for the input GEMM
SIG = 3 * HS                 # 576: sigmoid region width per strip
GEMM_AHEAD_TILES = 8         # GEMM M-tiles (8 timesteps each) emitted up front

# column permutation: ours -> original gate index
_PERM = np.zeros(G4, dtype=np.int64)
_BASE = {0: 0, 1: H, 2: 3 * H, 3: 2 * H}    # i, f, o, g
for _s in range(NSTRIP):
    for _blk in range(4):
        for _j in range(HS):
            _PERM[_s * WS + _blk * HS + _j] = _BASE[_blk] + HS * _s + _j


def build_nc(L_steps=L, interleave=True, stage=5):
    assert L_steps % 8 == 0
    nc = bacc.Bacc("TRN2", target_bir_lowering=False, debug=False)

    xT = nc.dram_tensor("xT", [I, L_steps, BC], F32R, kind="ExternalInput").ap()
    wihT = nc.dram_tensor("wihT", [I, G4], F32R, kind="ExternalInput").ap()
    whhS = nc.dram_tensor("whhS", [128, NC6, G4], BF16, kind="ExternalInput").ap()
    biasd = nc.dram_tensor("biasd", [1, G4], BF16, kind="ExternalInput").ap()
    h0S = nc.dram_tensor("h0S", [128, HS], BF16, kind="ExternalInput").ap()
    c0s = nc.dram_tensor("c0s", [NSTRIP, BC, HS], F32, kind="ExternalInput").ap()
    augd = nc.dram_tensor("augd", [17, 32], BF16, kind="ExternalInput").ap()
    hout = nc.dram_tensor("hout", [L_steps, BC, H], BF16, kind="ExternalOutput").ap()
    cnout = nc.dram_tensor("cnout", [NSTRIP, BC, HS], F32, kind="ExternalOutput").ap()

    MT = L_steps // 8            # GEMM M-tiles (128 rows = 8 t x 16 b)

    with ExitStack() as ctx:
        tc = ctx.enter_context(tile.TileContext(nc))
        wts = ctx.enter_context(tc.tile_pool(name="wts", bufs=1))
        state = ctx.enter_context(tc.tile_pool(name="state", bufs=1))
        xgp = ctx.enter_context(tc.tile_pool(name="xgp", bufs=3))
        work = ctx.enter_context(tc.tile_pool(name="work", bufs=2))
        hsp = ctx.enter_context(tc.tile_pool(name="hsp", bufs=2))
        gemmx = ctx.enter_context(tc.tile_pool(name="gemmx", bufs=3))
        gpsum = ctx.enter_context(tc.tile_pool(name="gpsum", bufs=2, space="PSUM"))
        gemmp = ctx.enter_context(tc.tile_pool(name="gemmp", bufs=2, space="PSUM"))
        dram = ctx.enter_context(tc.tile_pool(name="dram", bufs=1, space="DRAM"))

        # resident tensors
        wih_sb = wts.tile([128, KI, G4], F32R)
        whh_sb = wts.tile([128, NC6, G4], BF16)
        aug = wts.tile([17, 32], BF16)       # [I_16 pad to 32 ; ones]
        c = state.tile([128, HS], F32)       # cell state, strip s at rows 32s..

        nc.sync.dma_start(wih_sb[:], wihT.rearrange("(k p) g -> p k g", p=128))
        nc.sync.dma_start(whh_sb[:], whhS)
        nc.sync.dma_start(aug[:], augd)
        nc.vector.memset(c[:], 0.0)
        for s in range(NSTRIP):
            nc.sync.dma_start(c[32 * s:32 * s + BC, :], c0s[s])

        hTs = hsp.tile([128, HS], BF16, tag="hTs")   # block-scattered h^T
        nc.sync.dma_start(hTs[:], h0S)

        xg_dram = dram.tile([L_steps, BC, G4], BF16)

        # ---------------- phase A: input-projection GEMM ----------------
        gemm_jobs = []
        xt_tiles = {}

        def emit_gemm_block(m, n):
            if n == 0:
                t0 = 8 * m
                xt = gemmx.tile([128, KI, 128], F32R)
                nc.sync.dma_start(
                    xt[:], xT[:, t0:t0 + 8, :].rearrange("(k p) t b -> p k (t b)", p=128))
                xt_tiles[m] = xt
            xt = xt_tiles[m]
            ps = gemmp.tile([128, 512], F32)
            for k in range(KI):
                nc.tensor.matmul(ps[:], xt[:, k, :],
                                 wih_sb[:, k, 512 * n:512 * (n + 1)],
                                 start=(k == 0), stop=(k == KI - 1))
            xgs = gemmx.tile([128, 512], BF16, tag="xgs")
            nc.vector.tensor_copy(xgs[:], ps[:])
            nc.sync.dma_start(
                xg_dram[8 * m:8 * (m + 1), :, 512 * n:512 * (n + 1)], xgs[:])
            if n == G4 // 512 - 1:
                del xt_tiles[m]

        for m in range(MT):
            for n in range(G4 // 512):
                gemm_jobs.append((m, n))
        gemm_pos = 0

        def pump_gemm(target):
            nonlocal gemm_pos
            while gemm_pos < min(target, len(gemm_jobs)):
                emit_gemm_block(*gemm_jobs[gemm_pos])
                gemm_pos += 1

        nblk = G4 // 512
        if interleave:
            pump_gemm(GEMM_AHEAD_TILES * nblk)
        else:
            pump_gemm(len(gemm_jobs))

        # ---------------- phase B: recurrence ----------------
        for t in range(L_steps if stage >= 1 else 0):
            xg = xgp.tile([17, G4], BF16)
            nc.sync.dma_start(xg[0:BC, :], xg_dram[t])
            nc.sync.dma_start(xg[BC:BC + 1, :], biasd)

            g_ps = gpsum.tile([128, 1024], F32)
            # PSUM banks: nb0 -> psum cols [0:384] = gate cols [0:384] (i, f);
            # nb1 -> psum cols [512:896] = gate cols [384:768] (o, g).
            # Each (strip g, bank) accumulation group opens with start=True.
            for (po, go, nn) in [(0, 0, 384), (512, 384, 384)]:
                for g in range(NSTRIP):
                    nc.tensor.matmul(
                        g_ps[32 * g:32 * g + 32, po:po + nn],
                        aug[:], xg[:, WS * g + go:WS * g + go + nn],
                        start=True, stop=False,
                        tile_position=(0, 32 * g), skip_group_check=True)
            for (po, go, nn) in [(0, 0, 384), (512, 384, 384)]:
                for cc in range(NC6 if stage >= 2 else 0):
                    for g in range(NSTRIP):
                        nc.tensor.matmul(
                            g_ps[32 * g:32 * g + 32, po:po + nn],
                            hTs[:, 32 * cc:32 * cc + 32],
                            whh_sb[:, cc, WS * g + go:WS * g + go + nn],
                            start=False, stop=(cc == NC6 - 1),
                            tile_position=(0, 32 * g), skip_group_check=True)
                if po == 0 and stage >= 3:
                    # sigmoid(i, f) while the o,g bank is still streaming
                    sg = work.tile([128, SIG], F32)
                    nc.scalar.activation(sg[:, 0:384], g_ps[:, 0:384], AF.Sigmoid)

            if stage < 3:
                dg = work.tile([128, WS], BF16, tag="dg")
                nc.vector.tensor_copy(dg[:, :], g_ps[:, 0:WS])
                for s in range(NSTRIP):
                    nc.sync.dma_start(hout[t, :, HS * s:HS * (s + 1)],
                                      dg[32 * s:32 * s + BC, 0:HS])
                continue

            # psum layout: i [0:192], f [192:384], o [512:704], g [704:896]
            tg = work.tile([128, HS], F32)
            tc_ = work.tile([128, HS], F32)
            t1 = work.tile([128, HS], F32)
            h = work.tile([128, HS], BF16)
            nc.scalar.activation(tg[:, :], g_ps[:, 704:896], AF.Tanh)
            nc.vector.tensor_mul(c[:, :], sg[:, 192:384], c[:, :])
            nc.vector.tensor_mul(t1[:, :], sg[:, 0:192], tg[:, :])
            nc.scalar.activation(sg[:, 384:SIG], g_ps[:, 512:704], AF.Sigmoid)
            nc.vector.tensor_add(c[:, :], c[:, :], t1[:, :])
            nc.scalar.activation(tc_[:, :], c[:, :], AF.Tanh)
            nc.vector.tensor_mul(h[:, :], sg[:, 384:SIG], tc_[:, :])

            if stage >= 4:
                hTs = hsp.tile([128, HS], BF16, tag="hTs")
                nc.vector.transpose(hTs[:, 0:96], h[:, 0:96])
                nc.vector.transpose(hTs[:, 96:HS], h[:, 96:HS])

            for s in range(NSTRIP):
                nc.sync.dma_start(hout[t, :, HS * s:HS * (s + 1)],
                                  h[32 * s:32 * s + BC, :])
            if t == L_steps - 1:
                for s in range(NSTRIP):
                    nc.sync.dma_start(cnout[s], c[32 * s:32 * s + BC, :])

            if interleave:
                pump_gemm(GEMM_AHEAD_TILES * nblk + ((t + 1) * nblk * MT) // L_steps)
        pump_gemm(len(gemm_jobs))

    nc.compile()
    return nc


def make_in_maps(x, weight_ih, weight_hh, bias, weight_ih_reverse,
                 weight_hh_reverse, bias_reverse, h0, c0, L_steps=L):
    """Per-core input dicts. Core c: direction c//4, batch quarter c%4."""
    in_maps = []
    for core in range(8):
        d, q = core // 4, core % 4
        bsl = slice(BC * q, BC * (q + 1))
        xq = np.asarray(x[bsl, :L_steps], dtype=np.float32)
        if d == 1:
            xq = xq[:, ::-1, :]
        Wih = weight_ih if d == 0 else weight_ih_reverse
        Whh = weight_hh if d == 0 else weight_hh_reverse
        bb = bias if d == 0 else bias_reverse
        wihT_p = np.ascontiguousarray(np.asarray(Wih, np.float32).T[:, _PERM])
        whhT_p = np.asarray(Whh, np.float32).T[:, _PERM]     # (H, G4), rows = dims
        # scatter rows: whhS[32 s + j, c, :] = whhT_p[192 s + 32 c + j, :]
        whhS = np.ascontiguousarray(
            whhT_p.reshape(NSTRIP, NC6, 32, G4).transpose(0, 2, 1, 3)
            .reshape(128, NC6, G4)).astype(ml_dtypes.bfloat16)
        bias_p = np.ascontiguousarray(
            np.asarray(bb, np.float32)[_PERM])[None, :].astype(ml_dtypes.bfloat16)
        h0q = np.asarray(h0[d, bsl], np.float32)             # (BC, H)
        c0q = np.asarray(c0[d, bsl], np.float32)
        # initial block-scattered h^T: h0S[32 s + j, 32 c + b] = h0q[b, 192 s + 32 c + j]
        h0S = np.zeros((NSTRIP, 32, NC6, 32), np.float32)
        h0r = h0q.reshape(BC, NSTRIP, NC6, 32)               # [b, s, c, j]
        h0S[:, :, :, 0:BC] = h0r.transpose(1, 3, 2, 0)       # [s, j, c, b]
        h0S = h0S.reshape(128, HS).astype(ml_dtypes.bfloat16)
        c0_strips = np.ascontiguousarray(
            c0q.reshape(BC, NSTRIP, HS).transpose(1, 0, 2))  # (NSTRIP, BC, HS)
        aug_np = np.zeros((17, 32), ml_dtypes.bfloat16)
        aug_np[0:16, 0:BC] = np.eye(BC, dtype=np.float32)
        aug_np[16, :] = 1.0
        in_maps.append({
            "xT": np.ascontiguousarray(xq.transpose(2, 1, 0)),  # (I, L, BC)
            "wihT": wihT_p,
            "whhS": whhS,
            "biasd": bias_p,
            "h0S": h0S,
            "c0s": c0_strips,
            "augd": aug_np,
        })
    return in_maps


def assemble(results, L_steps=L):
    out_f = np.zeros((B, L_steps, H), np.float32)
    out_r = np.zeros((B, L_steps, H), np.float32)
    hn = np.zeros((2, B, H), np.float32)
    cn = np.zeros((2, B, H), np.float32)
    for core in range(8):
        d, q = core // 4, core % 4
        bsl = slice(BC * q, BC * (q + 1))
        hseq = np.asarray(results[core]["hout"],
                          dtype=np.float32).transpose(1, 0, 2)  # (BC, L, H)
        cfin = results[core]["cnout"].transpose(1, 0, 2).reshape(BC, H)
        if d == 0:
            out_f[bsl] = hseq
            hn[0, bsl] = hseq[:, -1, :]
        else:
            out_r[bsl] = hseq[:, ::-1, :]
            hn[1, bsl] = hseq[:, -1, :]
        cn[d, bsl] = cfin
    h_all = np.concatenate([out_f, out_r], axis=-1)
    return h_all, hn, cn


_NC_CACHE = {}


def _get_nc(L_steps=L, interleave=True):
    key = (L_steps, interleave)
    if key not in _NC_CACHE:
        _NC_CACHE[key] = build_nc(L_steps, interleave)
    return _NC_CACHE[key]


def kernel(x, weight_ih, weight_hh, bias, weight_ih_reverse,
           weight_hh_reverse, bias_reverse, h0, c0, _trace=False, _tmpdir=None):
    nc = _get_nc()
    in_maps = make_in_maps(x, weight_ih, weight_hh, bias, weight_ih_reverse,
                           weight_hh_reverse, bias_reverse, h0, c0)
    res = run_bass_kernel_spmd(nc, in_maps, core_ids=list(range(8)),
                               trace=_trace, tmpdir=_tmpdir)
    out = assemble(res.results)
    kernel.last_results = res
    return out
